# revision 24
# baseline (speedup 1.0000x reference)
"""Multi-head differential attention on 8 Trainium2 NeuronCores.

Sharding: core c -> batch c//4, head-group c%4 (4 of 16 heads).

Fully software-pipelined single-stream schedule per core:
 - weights DMA'd before x; the k0/q0 projection strips interleave their
   contraction-chunk matmuls with the xt chunk DMAs so the first softmax
   exp fires ~20us after the xt DMA completes;
 - attention is k-major (scores transposed; softmax denominators come
   from a ones column appended to each head's V block); lambda, the
   softmax scale and log2(e) are folded into the K strips (per-partition
   multiply on the PSUM drain), and the ACT exp uses scale=ln2 (2^x);
 - remaining q/k strips for both pairs are drip-fed one matmul at a time
   into the ACT-bound attention loop so they hide in PE slack;
 - per-qt post-processing is split: the AV-PSUM drain and the
   denominator DMA-broadcast roundtrip are issued immediately, but the
   dependent DVE chain (reciprocal/normalize/bn_stats) is deferred into
   the middle of the NEXT qt tile, so no engine FIFO head-of-line blocks;
 - the z AllGather is split into 4 per-(pair,half) chunks that overlap
   attention; GN scalars ride the last chunk as bitcast bf16 payload;
   gathered-chunk SBUF preloads are deferred 2 qt-steps so their DMA
   triggers never stall the sync queue on an in-flight collective;
 - warmup matmuls keep the PE HAM clock warm across the final
   gather wait; the out-projection folds the GN affine into a scaled Wo
   + constant bias row and streams per-512-column output tiles to HBM.
"""

import numpy as np
import ml_dtypes

B, S, D, H, DH = 2, 2048, 1024, 16, 64
HPC = 4            # heads per core
CW = HPC * DH      # attention columns per core (256)
EPS = 1e-5
LAMBDA_INIT = 0.8
N_CORES = 8
SCC = 16           # scalar payload columns (8 f32 as 16 bf16)
QT = 512           # query tile
NQT = 4
NKT = 16           # key tiles of 128
NDC = 8            # d-chunks of 128
VW = 65            # v block width per head (64 + ones column)

_cache = {}


def _build(with_collective=True):
    from contextlib import ExitStack
    import concourse.bass as bass
    from concourse import bacc
    import concourse.tile as tile
    import concourse.mybir as mybir

    f32 = mybir.dt.float32
    bf16 = mybir.dt.bfloat16
    AF = mybir.ActivationFunctionType
    ALU = mybir.AluOpType

    nc = bacc.Bacc("TRN2", target_bir_lowering=False, debug=False,
                   num_devices=N_CORES)

    xt_d = nc.dram_tensor("xt", [D, S], bf16, kind="ExternalInput")
    wq_d = nc.dram_tensor("wq", [128, NDC, CW], bf16, kind="ExternalInput")
    wk_d = nc.dram_tensor("wk", [128, NDC, CW], bf16, kind="ExternalInput")
    wv_d = nc.dram_tensor("wv", [128, NDC, CW], bf16, kind="ExternalInput")
    lamv_d = nc.dram_tensor("lamv", [128, 2], f32, kind="ExternalInput")
    wo_d = nc.dram_tensor("wo", [128, NDC, CW], bf16, kind="ExternalInput")
    bq_d = nc.dram_tensor("bq", [CW], bf16, kind="ExternalInput")
    bk_d = nc.dram_tensor("bk", [CW], bf16, kind="ExternalInput")
    bv_d = nc.dram_tensor("bv", [CW], f32, kind="ExternalInput")
    bvf_d = nc.dram_tensor("bvf", [D], f32, kind="ExternalInput")
    bo_d = nc.dram_tensor("bo", [CW], bf16, kind="ExternalInput")
    y_d = nc.dram_tensor("y", [2, 128, S], f32, kind="ExternalOutput")

    rs_d = nc.dram_tensor("rs_scratch", [HPC, S], f32)
    # per-(pair, half) gather chunks; the last one carries the GN scalars
    agi = {}
    ago = {}
    for t in range(2):
        for hf in range(2):
            w = 2 * QT + (SCC if (t == 1 and hf == 1) else 0)
            agi[t, hf] = nc.dram_tensor(f"agi{t}{hf}", [128, w], bf16)
            ago[t, hf] = nc.dram_tensor(f"ago{t}{hf}", [4, 128, w], bf16)

    groups = [[0, 1, 2, 3], [4, 5, 6, 7]]

    import os as _os
    with ExitStack() as ctx:
        tc = ctx.enter_context(tile.TileContext(nc))
        const = ctx.enter_context(tc.tile_pool(name="const", bufs=1))
        big = ctx.enter_context(tc.tile_pool(name="big", bufs=1))
        pd = ctx.enter_context(tc.tile_pool(name="pd", bufs=1))
        pexp = ctx.enter_context(tc.tile_pool(name="pexp", bufs=4))

        # ---- constants: weights FIRST so the first matmul starts early
        wq_sb = const.tile([128, NDC, CW], bf16, tag="wq")
        wk_sb = const.tile([128, NDC, CW], bf16, tag="wk")
        nc.sync.dma_start(out=wq_sb, in_=wq_d[:, :, :])
        nc.sync.dma_start(out=wk_sb, in_=wk_d[:, :, :])
        bqr_sb = const.tile([1, CW], bf16, tag="bqr")
        bkr_sb = const.tile([1, CW], bf16, tag="bkr")
        nc.sync.dma_start(out=bqr_sb, in_=bq_d[:].rearrange("(a n) -> a n", a=1))
        nc.sync.dma_start(out=bkr_sb, in_=bk_d[:].rearrange("(a n) -> a n", a=1))
        lamv_sb = const.tile([128, 2], f32, tag="lamv")
        nc.sync.dma_start(out=lamv_sb, in_=lamv_d[:, :])

        pxt = ctx.enter_context(tc.tile_pool(name="pxt", bufs=1))
        xt_sb = pxt.tile([128, NDC, S], bf16, tag="xt")
        for c in range(NDC):
            nc.sync.dma_start(out=xt_sb[:, c, :], in_=xt_d[c * 128:(c + 1) * 128, :])

        wv_sb = const.tile([128, NDC, CW], bf16, tag="wv")
        nc.sync.dma_start(out=wv_sb, in_=wv_d[:, :, :])
        wo_sb = const.tile([128, NDC, CW], bf16, tag="wo")
        nc.sync.dma_start(out=wo_sb, in_=wo_d[:, :, :])
        bor_sb = const.tile([1, CW], bf16, tag="bor")
        nc.sync.dma_start(out=bor_sb, in_=bo_d[:].rearrange("(a n) -> a n", a=1))
        bv0_sb = const.tile([64, HPC], f32, tag="bv0")
        nc.sync.dma_start(out=bv0_sb, in_=bv_d[:].rearrange("(h p) -> p h", p=64))
        bv_sb = const.tile([64, HPC], f32, tag="bv")
        nc.vector.tensor_copy(bv_sb, bv0_sb)  # pre-touch: keep deps DVE-local

        onesrow_sb = const.tile([1, QT], bf16, tag="onesrow")
        nc.vector.memset(onesrow_sb, 1.0)
        # dummy exp: pulls the ACT exp table load into the xt-DMA window
        tblw = pd.tile([1, 16], f32, tag="tblw", name="tblw")
        nc.scalar.activation(tblw, onesrow_sb[0:1, 0:16], AF.Exp)
        ones_sb = const.tile([64, 1], f32, tag="ones")
        nc.vector.memset(ones_sb, 1.0)
        ones2_sb = const.tile([2, 64], f32, tag="ones2")
        nc.vector.memset(ones2_sb, 1.0)

        qT_sb = big.tile([128, 2, S], bf16, tag="qT")   # pair t: head 2t rows 0:64
        kT_sb = big.tile([128, 2, S], bf16, tag="kT")
        # v blocks: head h at cols [65h, 65h+64), ones col at 65h+64
        v2_sb = big.tile([128, NKT, HPC * VW], bf16, tag="v2")
        nc.vector.memset(v2_sb, 1.0)
        zp_sb = [big.tile([128, S + SCC], bf16, tag=f"zp{t}", name=f"zp{t}")
                 for t in range(2)]
        nrmg_sb = big.tile([128, NDC, S], bf16, tag="nrmg")

        # per-head GN stats staging
        bnst = [pd.tile([64, NQT, 6], f32, tag=f"bn{h}", name=f"bnst{h}")
                for h in range(HPC)]
        vr_all = pd.tile([1, HPC], f32, tag="vr_all", name="vr_all")
        msc_all = pd.tile([1, 2 * HPC], f32, tag="msc", name="msc_all")
        stk_big = pd.tile([64, HPC, 3], f32, tag="stk", name="stk_big")
        stk_all = [stk_big[:, j, :] for j in range(HPC)]

        from contextlib import ExitStack as _ES
        cctx = _ES()
        psc = cctx.enter_context(tc.tile_pool(name="psc", bufs=2, space="PSUM"))
        pav = cctx.enter_context(tc.tile_pool(name="pav", bufs=2, space="PSUM"))
        pbqk = cctx.enter_context(tc.tile_pool(name="pbqk", bufs=2, space="PSUM"))

        # lazily-emitted projection strips, spread across the attention loop
        strip_queue = []


        def push_strip(t, w_sb, br_sb, dst, st):
            strip_queue.append({"t": t, "w": w_sb, "br": br_sb, "dst": dst,
                                "st": st, "c": 0, "ps": None,
                                "name": f"kq{t}{dst.tensor.name}{st}"})

        def strip_step(nmm):
            while nmm > 0 and strip_queue:
                s = strip_queue[0]
                if s["ps"] is None:
                    s["ps"] = pbqk.tile([128, QT], f32, tag="qk", name=s["name"])
                t, st = s["t"], s["st"]
                if s["c"] < NDC:
                    c = s["c"]
                    nc.tensor.matmul(s["ps"],
                                     s["w"][:, c, t * 128:(t + 1) * 128],
                                     xt_sb[:, c, st * QT:(st + 1) * QT],
                                     start=(c == 0), stop=False)
                    s["c"] += 1
                else:
                    nc.tensor.matmul(s["ps"], s["br"][:, t * 128:(t + 1) * 128],
                                     onesrow_sb, start=False, stop=True)
                    if s["w"] is wk_sb:
                        # scores scale: lambda(head) * DH^-0.5 folded into K
                        nc.vector.tensor_scalar(
                            out=s["dst"][:, t, st * QT:(st + 1) * QT],
                            in0=s["ps"], scalar1=lamv_sb[:, t:t + 1],
                            scalar2=None, op0=ALU.mult)
                    else:
                        nc.vector.tensor_copy(
                            out=s["dst"][:, t, st * QT:(st + 1) * QT], in_=s["ps"])
                    strip_queue.pop(0)
                nmm -= 1

        def kq_strip(t, w_sb, br_sb, dst, st):
            push_strip(t, w_sb, br_sb, dst, st)
            strip_step(10 ** 6)

        def v_group(kt):
            # v strips kt and kt+1 in one PSUM tile, one strided copy out
            ps = pbqk.tile([128, QT], f32, tag="qk", name=f"vg{kt}")
            # j-outer: each accumulation group completes before the next
            # starts (start=True clears has_written for the WHOLE bank)
            for j in range(2):
                for c in range(NDC):
                    nc.tensor.matmul(ps[:, j * CW:(j + 1) * CW],
                                     xt_sb[:, c,
                                           (kt + j) * 128:(kt + j + 1) * 128],
                                     wv_sb[:, c, :],
                                     start=(c == 0), stop=(c == NDC - 1))
            nc.vector.tensor_copy(
                out=v2_sb[:, kt:kt + 2, :]
                    .rearrange("p k (h x) -> p k h x", x=VW)[:, :, :, 0:DH],
                in_=ps.rearrange("p (k h x) -> p k h x", k=2, x=DH))

        # minimal pair-0 prefix: k strip 0 + q strip 0, interleaved per
        # d-chunk so both finish right after the last xt chunk DMA lands
        psk = pbqk.tile([128, QT], f32, tag="qk", name="pfx_k")
        psq = pbqk.tile([128, QT], f32, tag="qk", name="pfx_q")
        for c in range(NDC):
            nc.tensor.matmul(psk, wk_sb[:, c, 0:128], xt_sb[:, c, 0:QT],
                             start=(c == 0), stop=False)
            nc.tensor.matmul(psq, wq_sb[:, c, 0:128], xt_sb[:, c, 0:QT],
                             start=(c == 0), stop=False)
        nc.tensor.matmul(psk, bkr_sb[:, 0:128], onesrow_sb, start=False, stop=True)
        nc.tensor.matmul(psq, bqr_sb[:, 0:128], onesrow_sb, start=False, stop=True)
        nc.vector.tensor_scalar(out=kT_sb[:, 0, 0:QT], in0=psk,
                                scalar1=lamv_sb[:, 0:1], scalar2=None,
                                op0=ALU.mult)
        nc.vector.tensor_copy(out=qT_sb[:, 0, 0:QT], in_=psq)

        # strips to hide under each (t, qt) attention tile
        plan = {
            (0, 1): [(1, wk_sb, bkr_sb, kT_sb, 0), (1, wq_sb, bqr_sb, qT_sb, 0),
                     (0, wq_sb, bqr_sb, qT_sb, 2)],
            (0, 2): [(1, wk_sb, bkr_sb, kT_sb, 1), (1, wk_sb, bkr_sb, kT_sb, 2),
                     (0, wq_sb, bqr_sb, qT_sb, 3)],
            (0, 3): [(1, wk_sb, bkr_sb, kT_sb, 3), (1, wq_sb, bqr_sb, qT_sb, 1)],
            (1, 0): [(1, wq_sb, bqr_sb, qT_sb, 2)],
            (1, 1): [(1, wq_sb, bqr_sb, qT_sb, 3)],
        }

        pending_post = [None]
        pending_nrmg = []
        step_ctr = [0]

        def flush_nrmg(min_age):
            for item in list(pending_nrmg):
                if step_ctr[0] - item[0] >= min_age:
                    _, tt, hf, hsl = item
                    for g in range(4):
                        nc.sync.dma_start(out=nrmg_sb[:, 2 * g + tt, hsl],
                                          in_=ago[tt, hf][g])
                    pending_nrmg.remove(item)

        def make_post(t, qt, h0, h1, zs0, zs1, rb0, rb1):
            qsl = slice(qt * QT, (qt + 1) * QT)

            def post():
                nc.vector.reciprocal_approx_fast(rb0, rb0)
                nc.vector.reciprocal_approx_fast(rb1, rb1)
                nc.vector.tensor_mul(zp_sb[t][0:64, qsl], zs0[0:DH, :], rb0)
                zst = pd.tile([64, QT], bf16, tag="zst", bufs=2,
                              name=f"zst{t}{qt}")
                nc.vector.tensor_mul(zst, zs1[0:DH, :], rb1)
                nc.gpsimd.dma_start(out=zp_sb[t][64:128, qsl], in_=zst)
                nc.vector.bn_stats(out=bnst[2 * (h0 % 2) + h0 // 2][:, qt, :],
                                   in_=zp_sb[t][0:64, qsl])
                nc.vector.bn_stats(out=bnst[2 * (h1 % 2) + h1 // 2][:, qt, :],
                                   in_=zst)
                # gather half-chunk once both its qts are posted
                if qt % 2 == 1 and not (t == 1 and qt == 3):
                    hf = qt // 2
                    hsl = slice(hf * 2 * QT, (hf + 1) * 2 * QT)
                    nc.sync.dma_start(out=agi[t, hf][:, :], in_=zp_sb[t][:, hsl])
                    if with_collective:
                        nc.gpsimd.collective_compute(
                            "AllGather", ALU.bypass, replica_groups=groups,
                            ins=[agi[t, hf][:].opt()], outs=[ago[t, hf][:].opt()])
                    else:
                        for g in range(4):
                            nc.sync.dma_start(out=ago[t, hf][g],
                                              in_=agi[t, hf][:, :])
                    pending_nrmg.append([step_ctr[0], t, hf, hsl])
                if qt == 3:
                    # pair-end stats aggregation (parity-major slots)
                    for h in (h0, h1):
                        j = 2 * (h % 2) + h // 2
                        mvh = pd.tile([64, 2], f32, tag="mv", bufs=2,
                                      name=f"mv{h}")
                        nc.vector.bn_aggr(out=mvh, in_=bnst[j])
                        stk = stk_all[j]
                        nc.vector.tensor_add(stk[:, 0:1], mvh[:, 0:1],
                                             bv_sb[:, h:h + 1])
                        nc.vector.tensor_copy(stk[:, 1:2], mvh[:, 1:2])
                        nc.vector.tensor_mul(stk[:, 2:3], stk[:, 0:1],
                                             stk[:, 0:1])

            return post

        for t in range(2):
            h0, h1 = 2 * t, 2 * t + 1
            for qt in range(NQT):
                step_ctr[0] += 1
                flush_nrmg(2)
                # safety: nothing stale may remain queued once this tile's
                # scores (which may read strip outputs) are emitted
                strip_step(10 ** 6)
                for sp in plan.get((t, qt), []):
                    push_strip(*sp)
                qsl = slice(qt * QT, (qt + 1) * QT)
                av0 = pav.tile([VW, QT], f32, tag="av", name=f"av{t}{qt}a")
                av1 = pav.tile([VW, QT], f32, tag="av", name=f"av{t}{qt}b")
                for kt in range(16):
                    sps = psc.tile([128, 2 * QT], f32, tag="s", name=f"s{t}{qt}{kt}")
                    for o in range(2):
                        nc.tensor.matmul(
                            sps[:, o * QT:(o + 1) * QT],
                            kT_sb[64 * o:64 * (o + 1), t, kt * 128:(kt + 1) * 128],
                            qT_sb[64 * o:64 * (o + 1), t, qt * QT:(qt + 1) * QT],
                            start=True, stop=True)
                    e_sb = pexp.tile([128, 2 * QT], bf16, tag="e", name=f"e{t}{qt}{kt}")
                    if kt % 3 == 1 and _os.environ.get("DVEEXP", "0") == "1":
                        # Schraudolph 2^x in bf16 via int16 bit trick (DVE)
                        nc.vector.tensor_scalar(
                            out=e_sb[:, :].bitcast(mybir.dt.int16), in0=sps,
                            scalar1=128.0, scalar2=16250.5,
                            op0=ALU.mult, op1=ALU.add)
                    else:
                        nc.scalar.activation(e_sb, sps, AF.Exp,
                                             scale=0.6931471805599453)
                    if t == 0 and qt == 0:
                        if kt % 2 == 0:
                            v_group(kt)
                        if kt in (2, 6, 10):
                            kq_strip(0, wk_sb, bkr_sb, kT_sb, kt // 4 + 1)
                        if kt == 8:
                            push_strip(0, wq_sb, bqr_sb, qT_sb, 1)
                    if kt == 6 and pending_post[0] is not None:
                        pending_post[0]()
                        pending_post[0] = None
                    nc.tensor.matmul(av0, v2_sb[:, kt, h0 * VW:(h0 + 1) * VW],
                                     e_sb[:, 0:QT],
                                     start=(kt == 0), stop=(kt == NKT - 1))
                    nc.tensor.matmul(av1, v2_sb[:, kt, h1 * VW:(h1 + 1) * VW],
                                     e_sb[:, QT:2 * QT],
                                     start=(kt == 0), stop=(kt == NKT - 1))
                    strip_step(2)

                if pending_post[0] is not None:
                    pending_post[0]()
                    pending_post[0] = None
                # immediate part of the qt post: drain the AV PSUM and kick
                # off the denominator-broadcast DMA roundtrip in background
                zs0 = pd.tile([VW, QT], f32, tag="zs", bufs=4, name=f"zs{t}{qt}a")
                zs1 = pd.tile([VW, QT], f32, tag="zs", bufs=4, name=f"zs{t}{qt}b")
                nc.vector.tensor_copy(zs0, av0)
                nc.vector.tensor_copy(zs1, av1)
                nc.sync.dma_start(out=rs_d[h0:h0 + 1, qsl], in_=zs0[DH:DH + 1, :])
                nc.sync.dma_start(out=rs_d[h1:h1 + 1, qsl], in_=zs1[DH:DH + 1, :])
                rb0 = pd.tile([64, QT], f32, tag="rb", bufs=4, name=f"rb{t}{qt}a")
                rb1 = pd.tile([64, QT], f32, tag="rb", bufs=4, name=f"rb{t}{qt}b")
                nc.gpsimd.dma_start(
                    out=rb0, in_=rs_d[h0:h0 + 1, qsl].to_broadcast([64, QT]))
                nc.gpsimd.dma_start(
                    out=rb1, in_=rs_d[h1:h1 + 1, qsl].to_broadcast([64, QT]))
                pending_post[0] = make_post(t, qt, h0, h1, zs0, zs1, rb0, rb1)

        # flush the last post (pair-1 qt3) immediately — it is the tail path
        pending_post[0]()
        pending_post[0] = None
        step_ctr[0] += 2
        flush_nrmg(0)

        # ---- GN scalar tail (after last exp; one sqrt table switch) ----
        stp = pbqk.tile([128, QT], f32, tag="qk", name="stp")
        nc.tensor.matmul(stp[0:1, 0:12], ones_sb, stk_big[:, :, :],
                         start=True, stop=True)
        e3 = pd.tile([1, HPC, 3], f32, tag="e3", name="e3")
        nc.vector.tensor_scalar(
            out=e3, in0=stp[0:1, 0:12].rearrange("p (h x) -> p h x", x=3),
            scalar1=1.0 / 64.0, scalar2=None, op0=ALU.mult)
        m2 = pd.tile([1, HPC], f32, tag="m2", name="m2")
        nc.vector.tensor_mul(m2, e3[:, :, 0], e3[:, :, 0])
        nc.vector.tensor_add(vr_all, e3[:, :, 1], e3[:, :, 2])
        nc.vector.tensor_tensor(out=vr_all, in0=vr_all, in1=m2, op=ALU.subtract)
        eps_t = pd.tile([1, 1], f32, tag="eps", name="eps_t")
        nc.vector.memset(eps_t, EPS)
        sd_all = pd.tile([1, HPC], f32, tag="sd", name="sd_all")
        nc.scalar.activation(sd_all, vr_all, AF.Sqrt, bias=eps_t)
        rr = pd.tile([1, HPC], f32, tag="rr", name="rr")
        nc.vector.reciprocal(rr, sd_all)
        # stats already parity-major: payload = [M0,M2,M1,M3, r0,r2,r1,r3]
        nc.vector.tensor_copy(msc_all[:, 0:HPC], e3[:, :, 0])
        nc.vector.tensor_copy(msc_all[:, HPC:2 * HPC], rr)

        # scalars ride the last gather chunk as bitcast bf16 payload columns
        warm = pbqk.tile([128, QT], f32, tag="qk", name="warm")
        for i in range(70):
            nc.tensor.matmul(warm, wo_sb[:, i % NDC, 0:128],
                             qT_sb[:, 0, 0:QT], start=True, stop=True)

        nc.vector.tensor_copy(out=zp_sb[1][0:1, S:S + SCC],
                              in_=msc_all[0:1, :].bitcast(bf16))
        nc.sync.dma_start(out=agi[1, 1][:, :], in_=zp_sb[1][:, 2 * QT:S + SCC])
        if with_collective:
            nc.gpsimd.collective_compute(
                "AllGather", ALU.bypass, replica_groups=groups,
                ins=[agi[1, 1][:].opt()], outs=[ago[1, 1][:].opt()])
        else:
            for g in range(4):
                nc.sync.dma_start(out=ago[1, 1][g], in_=agi[1, 1][:, :])
        for g in range(4):
            nc.sync.dma_start(out=nrmg_sb[:, 2 * g + 1, 2 * QT:S],
                              in_=ago[1, 1][g, :, 0:2 * QT])

        cctx.close()

        # ---- fold GN affine into Wo; column-parallel out-projection ----
        with tc.tile_pool(name="pg", bufs=1) as pg, \
             tc.tile_pool(name="pf", bufs=4, space="PSUM") as pf, \
             tc.tile_pool(name="pystage", bufs=4) as pystage:
            # gathered scalars: [4 groups, 8 f32] as bitcast bf16 rows
            sc16 = pg.tile([1, 4, SCC], bf16, tag="sc16")
            nc.sync.dma_start(
                out=sc16,
                in_=ago[1, 1][:, 0:1, 2 * QT:2 * QT + SCC].rearrange("g p c -> p g c"))
            scf = sc16[:, :, :].bitcast(f32)   # [1, 4, 8]: (M0,M2,M1,M3, r0,r2,r1,r3)
            s2p = pf.tile([128, NDC], f32, tag="sp", bufs=1, name="s2p")
            mcp = pf.tile([128, NDC], f32, tag="mc", bufs=1, name="mcp")
            for o in range(2):
                nc.tensor.matmul(s2p[64 * o:64 * (o + 1), :], ones2_sb[0:1, :],
                                 scf[:, :, HPC + 2 * o:HPC + 2 * o + 2],
                                 start=True, stop=True)
                nc.tensor.matmul(mcp[64 * o:64 * (o + 1), :], ones2_sb[0:1, :],
                                 scf[:, :, 2 * o:2 * o + 2],
                                 start=True, stop=True)
            s2c = pg.tile([128, NDC], f32, tag="s2c")
            nc.vector.tensor_copy(s2c, s2p)
            bvg = pg.tile([128, NDC], f32, tag="bvg")
            nc.sync.dma_start(out=bvg, in_=bvf_d[:].rearrange("(c p) -> p c", p=128))
            mcs = pg.tile([128, NDC], f32, tag="mcs")
            nc.vector.tensor_tensor(out=mcs, in0=mcp, in1=bvg, op=ALU.subtract)
            mvec = pg.tile([128, NDC], bf16, tag="mvec")
            nc.vector.tensor_mul(mvec, mcs, s2c)

            # wo_scaled[p, (c,n)] = r_head(p,c) * wo ; cst[n] = sum_p M*r*wo
            wos = pg.tile([128, NDC, CW], bf16, tag="wos")
            for c in range(NDC):
                nc.vector.tensor_scalar(out=wos[:, c, :], in0=wo_sb[:, c, :],
                                        scalar1=s2c[:, c:c + 1], scalar2=None,
                                        op0=ALU.mult)
            cstp = pf.tile([128, QT], f32, tag="cs", bufs=1, name="cstp")
            for c in range(NDC):
                nc.tensor.matmul(cstp[0:1, 0:CW], mvec[:, c:c + 1], wo_sb[:, c, :],
                                 start=(c == 0), stop=(c == NDC - 1))
            brow = pg.tile([1, CW], bf16, tag="brow")
            nc.vector.tensor_tensor(out=brow, in0=bor_sb, in1=cstp[0:1, 0:CW],
                                    op=ALU.subtract)

            # per-st pipelined out-projection: 8 chunk matmuls + bias row
            for st in range(NQT):
                ssl = slice(st * QT, (st + 1) * QT)
                for nt in range(2):
                    yps = pf.tile([128, QT], f32, tag="yp", name=f"yp{nt}{st}")
                    for c in range(NDC):
                        nc.tensor.matmul(yps, wos[:, c, nt * 128:(nt + 1) * 128],
                                         nrmg_sb[:, c, ssl],
                                         start=(c == 0), stop=False)
                    nc.tensor.matmul(yps, brow[:, nt * 128:(nt + 1) * 128],
                                     onesrow_sb, start=False, stop=True)
                    ystage = pystage.tile([128, QT], f32, tag="ys",
                                          name=f"ys{nt}{st}")
                    nc.vector.tensor_copy(ystage, yps)
                    nc.sync.dma_start(out=y_d[nt, :, ssl], in_=ystage)

    nc.compile()
    return nc


def _get_nc():
    if "nc" not in _cache:
        _cache["nc"] = _build()
    return _cache["nc"]


def _host_prep(x, Wq, bq, Wk, bk, Wv, bv, Wo, bo, lq1, lk1, lq2, lk2, gn_w, gn_b):
    x = np.asarray(x, np.float32)
    lam = (np.exp((np.asarray(lq1) * np.asarray(lk1)).sum(-1))
           - np.exp((np.asarray(lq2) * np.asarray(lk2)).sum(-1)) + LAMBDA_INIT)
    qscale = (DH ** -0.5) * lam * np.log2(np.e)
    gw = np.asarray(gn_w).reshape(D)
    gb = np.asarray(gn_b).reshape(D)
    Wo_eff = np.asarray(Wo) * gw[:, None]
    bo_eff = np.asarray(bo) + gb @ np.asarray(Wo)

    # Gathered-row order (chunk (g t), partition (o,dh) -> head 4g+2t+o) is
    # exactly the original row-major head order, so Wo_eff rows need no
    # permutation.
    xT = np.ascontiguousarray(x.transpose(0, 2, 1))  # [B, D, S]
    bf = ml_dtypes.bfloat16

    in_maps = []
    for c in range(N_CORES):
        b, hg = c // 4, c % 4
        cs = slice(CW * hg, CW * (hg + 1))
        lamv = np.empty((128, 2), np.float32)
        for t in range(2):
            lamv[0:64, t] = qscale[4 * hg + 2 * t]
            lamv[64:128, t] = qscale[4 * hg + 2 * t + 1]
        in_maps.append({
            "xt": np.ascontiguousarray(xT[b]).astype(bf),
            "wq": _wlayout(np.asarray(Wq)[:, cs]).astype(bf),
            "wk": _wlayout(np.asarray(Wk)[:, cs]).astype(bf),
            "wv": _wlayout(np.asarray(Wv)[:, cs]).astype(bf),
            "lamv": lamv,
            "wo": _wlayout(Wo_eff[:, cs]).astype(bf),
            "bq": np.ascontiguousarray(np.asarray(bq)[cs]).astype(bf),
            "bk": np.ascontiguousarray(np.asarray(bk)[cs]).astype(bf),
            "bv": np.ascontiguousarray(np.asarray(bv)[cs]).astype(np.float32),
            "bvf": np.ascontiguousarray(np.asarray(bv)).astype(np.float32),
            "bo": np.ascontiguousarray(bo_eff[cs]).astype(bf),
        })
    return in_maps


def _wlayout(w):
    # [D, CW] -> [128, NDC, CW] so the on-device weight DMA is contiguous
    return np.ascontiguousarray(w.reshape(NDC, 128, CW).transpose(1, 0, 2))


def _host_gather(outs):
    # core c=4b+hg produced output columns [256*hg, 256*(hg+1)) as [2,128,S]
    yT = np.empty((B, D, S), np.float32)
    for b in range(B):
        for hg in range(4):
            q = np.asarray(outs[4 * b + hg]["y"]).reshape(CW, S)
            yT[b, CW * hg:CW * (hg + 1), :] = q
    return np.ascontiguousarray(yT.transpose(0, 2, 1))


def kernel(x, Wq, bq, Wk, bk, Wv, bv, Wo, bo, lq1, lk1, lq2, lk2, gn_w, gn_b):
    from concourse.bass_utils import run_bass_kernel_spmd

    in_maps = _host_prep(x, Wq, bq, Wk, bk, Wv, bv, Wo, bo,
                         lq1, lk1, lq2, lk2, gn_w, gn_b)
    nc = _get_nc()
    res = run_bass_kernel_spmd(nc, in_maps, core_ids=list(range(N_CORES)))
    return _host_gather(res.results)


# revision 25
# speedup vs baseline: 1.0353x; 1.0353x over previous
"""Multi-head differential attention on 8 Trainium2 NeuronCores.

Sharding: core c -> batch c//4, head-group c%4 (4 of 16 heads).

Fully software-pipelined single-stream schedule per core:
 - weights DMA'd before x; the k0/q0 projection strips interleave their
   contraction-chunk matmuls with the xt chunk DMAs so the first softmax
   exp fires ~20us after the xt DMA completes;
 - attention is k-major (scores transposed; softmax denominators come
   from a ones column appended to each head's V block); lambda, the
   softmax scale and log2(e) are folded into the K strips (per-partition
   multiply on the PSUM drain), and the ACT exp uses scale=ln2 (2^x);
 - remaining q/k strips for both pairs are drip-fed one matmul at a time
   into the ACT-bound attention loop so they hide in PE slack;
 - per-qt post-processing is split: the AV-PSUM drain and the
   denominator DMA-broadcast roundtrip are issued immediately, but the
   dependent DVE chain (reciprocal/normalize/bn_stats) is deferred into
   the middle of the NEXT qt tile, so no engine FIFO head-of-line blocks;
 - the z AllGather is split into 4 per-(pair,half) chunks that overlap
   attention; GN scalars ride the last chunk as bitcast bf16 payload;
   gathered-chunk SBUF preloads are deferred 2 qt-steps so their DMA
   triggers never stall the sync queue on an in-flight collective;
 - warmup matmuls keep the PE HAM clock warm across the final
   gather wait; the out-projection folds the GN affine into a scaled Wo
   + constant bias row and streams per-512-column output tiles to HBM.
"""

import numpy as np
import ml_dtypes

B, S, D, H, DH = 2, 2048, 1024, 16, 64
HPC = 4            # heads per core
CW = HPC * DH      # attention columns per core (256)
EPS = 1e-5
LAMBDA_INIT = 0.8
N_CORES = 8
SCC = 16           # scalar payload columns (8 f32 as 16 bf16)
QT = 512           # query tile
NQT = 4
NKT = 16           # key tiles of 128
NDC = 8            # d-chunks of 128
VW = 65            # v block width per head (64 + ones column)

_cache = {}


def _build(with_collective=True):
    from contextlib import ExitStack
    import concourse.bass as bass
    from concourse import bacc
    import concourse.tile as tile
    import concourse.mybir as mybir

    f32 = mybir.dt.float32
    bf16 = mybir.dt.bfloat16
    AF = mybir.ActivationFunctionType
    ALU = mybir.AluOpType

    nc = bacc.Bacc("TRN2", target_bir_lowering=False, debug=False,
                   num_devices=N_CORES)

    xt_d = nc.dram_tensor("xt", [D, S], bf16, kind="ExternalInput")
    wq_d = nc.dram_tensor("wq", [128, NDC, CW], bf16, kind="ExternalInput")
    wk_d = nc.dram_tensor("wk", [128, NDC, CW], bf16, kind="ExternalInput")
    wv_d = nc.dram_tensor("wv", [128, NDC, CW], bf16, kind="ExternalInput")
    lamv_d = nc.dram_tensor("lamv", [128, 2], f32, kind="ExternalInput")
    wo_d = nc.dram_tensor("wo", [128, NDC, CW], bf16, kind="ExternalInput")
    bq_d = nc.dram_tensor("bq", [CW], bf16, kind="ExternalInput")
    bk_d = nc.dram_tensor("bk", [CW], bf16, kind="ExternalInput")
    bv_d = nc.dram_tensor("bv", [CW], f32, kind="ExternalInput")
    bvf_d = nc.dram_tensor("bvf", [D], f32, kind="ExternalInput")
    bo_d = nc.dram_tensor("bo", [CW], bf16, kind="ExternalInput")
    y_d = nc.dram_tensor("y", [2, 128, S], f32, kind="ExternalOutput")

    rs_d = nc.dram_tensor("rs_scratch", [HPC, S], f32)
    # per-(pair, half) gather chunks; the last one carries the GN scalars
    agi = {}
    ago = {}
    for t in range(2):
        for hf in range(2):
            w = 2 * QT + (SCC if (t == 1 and hf == 1) else 0)
            agi[t, hf] = nc.dram_tensor(f"agi{t}{hf}", [128, w], bf16)
            ago[t, hf] = nc.dram_tensor(f"ago{t}{hf}", [4, 128, w], bf16)

    groups = [[0, 1, 2, 3], [4, 5, 6, 7]]

    import os as _os
    with ExitStack() as ctx:
        tc = ctx.enter_context(tile.TileContext(nc))
        const = ctx.enter_context(tc.tile_pool(name="const", bufs=1))
        big = ctx.enter_context(tc.tile_pool(name="big", bufs=1))
        pd = ctx.enter_context(tc.tile_pool(name="pd", bufs=1))
        pexp = ctx.enter_context(tc.tile_pool(name="pexp", bufs=4))

        # ---- constants: weights FIRST so the first matmul starts early
        wq_sb = const.tile([128, NDC, CW], bf16, tag="wq")
        wk_sb = const.tile([128, NDC, CW], bf16, tag="wk")
        nc.sync.dma_start(out=wq_sb, in_=wq_d[:, :, :])
        nc.sync.dma_start(out=wk_sb, in_=wk_d[:, :, :])
        bqr_sb = const.tile([1, CW], bf16, tag="bqr")
        bkr_sb = const.tile([1, CW], bf16, tag="bkr")
        nc.sync.dma_start(out=bqr_sb, in_=bq_d[:].rearrange("(a n) -> a n", a=1))
        nc.sync.dma_start(out=bkr_sb, in_=bk_d[:].rearrange("(a n) -> a n", a=1))
        lamv_sb = const.tile([128, 2], f32, tag="lamv")
        nc.sync.dma_start(out=lamv_sb, in_=lamv_d[:, :])

        pxt = ctx.enter_context(tc.tile_pool(name="pxt", bufs=1))
        xt_sb = pxt.tile([128, NDC, S], bf16, tag="xt")
        for c in range(NDC):
            nc.sync.dma_start(out=xt_sb[:, c, :], in_=xt_d[c * 128:(c + 1) * 128, :])

        wv_sb = const.tile([128, NDC, CW], bf16, tag="wv")
        nc.sync.dma_start(out=wv_sb, in_=wv_d[:, :, :])
        wo_sb = const.tile([128, NDC, CW], bf16, tag="wo")
        nc.sync.dma_start(out=wo_sb, in_=wo_d[:, :, :])
        bor_sb = const.tile([1, CW], bf16, tag="bor")
        nc.sync.dma_start(out=bor_sb, in_=bo_d[:].rearrange("(a n) -> a n", a=1))
        bv0_sb = const.tile([64, HPC], f32, tag="bv0")
        nc.sync.dma_start(out=bv0_sb, in_=bv_d[:].rearrange("(h p) -> p h", p=64))
        bv_sb = const.tile([64, HPC], f32, tag="bv")
        nc.vector.tensor_copy(bv_sb, bv0_sb)  # pre-touch: keep deps DVE-local

        onesrow_sb = const.tile([1, QT], bf16, tag="onesrow")
        nc.vector.memset(onesrow_sb, 1.0)
        # dummy exp: pulls the ACT exp table load into the xt-DMA window
        tblw = pd.tile([1, 16], f32, tag="tblw", name="tblw")
        nc.scalar.activation(tblw, onesrow_sb[0:1, 0:16], AF.Exp)
        ones_sb = const.tile([64, 1], f32, tag="ones")
        nc.vector.memset(ones_sb, 1.0)
        ones2_sb = const.tile([2, 64], f32, tag="ones2")
        nc.vector.memset(ones2_sb, 1.0)

        qT_sb = big.tile([128, 2, S], bf16, tag="qT")   # pair t: head 2t rows 0:64
        kT_sb = big.tile([128, 2, S], bf16, tag="kT")
        # v blocks: head h at cols [65h, 65h+64), ones col at 65h+64
        v2_sb = big.tile([128, NKT, HPC * VW], bf16, tag="v2")
        nc.vector.memset(v2_sb, 1.0)
        zp_sb = [big.tile([128, S + SCC], bf16, tag=f"zp{t}", name=f"zp{t}")
                 for t in range(2)]
        nrmg_sb = big.tile([128, NDC, S], bf16, tag="nrmg")

        # per-head GN stats staging
        bnst = [pd.tile([64, NQT, 6], f32, tag=f"bn{h}", name=f"bnst{h}")
                for h in range(HPC)]
        vr_all = pd.tile([1, HPC], f32, tag="vr_all", name="vr_all")
        msc_all = pd.tile([1, 2 * HPC], f32, tag="msc", name="msc_all")
        stk_big = pd.tile([64, HPC, 3], f32, tag="stk", name="stk_big")
        stk_all = [stk_big[:, j, :] for j in range(HPC)]

        from contextlib import ExitStack as _ES
        cctx = _ES()
        psc = cctx.enter_context(tc.tile_pool(name="psc", bufs=2, space="PSUM"))
        pav = cctx.enter_context(tc.tile_pool(name="pav", bufs=2, space="PSUM"))
        pbqk = cctx.enter_context(tc.tile_pool(name="pbqk", bufs=2, space="PSUM"))

        # lazily-emitted projection strips, spread across the attention loop
        strip_queue = []


        def push_strip(t, w_sb, br_sb, dst, st):
            strip_queue.append({"t": t, "w": w_sb, "br": br_sb, "dst": dst,
                                "st": st, "c": 0, "ps": None,
                                "name": f"kq{t}{dst.tensor.name}{st}"})

        def strip_step(nmm):
            while nmm > 0 and strip_queue:
                s = strip_queue[0]
                if s["ps"] is None:
                    s["ps"] = pbqk.tile([128, QT], f32, tag="qk", name=s["name"])
                t, st = s["t"], s["st"]
                if s["c"] < NDC:
                    c = s["c"]
                    nc.tensor.matmul(s["ps"],
                                     s["w"][:, c, t * 128:(t + 1) * 128],
                                     xt_sb[:, c, st * QT:(st + 1) * QT],
                                     start=(c == 0), stop=False)
                    s["c"] += 1
                else:
                    nc.tensor.matmul(s["ps"], s["br"][:, t * 128:(t + 1) * 128],
                                     onesrow_sb, start=False, stop=True)
                    if s["w"] is wk_sb:
                        # scores scale: lambda(head) * DH^-0.5 folded into K
                        nc.vector.tensor_scalar(
                            out=s["dst"][:, t, st * QT:(st + 1) * QT],
                            in0=s["ps"], scalar1=lamv_sb[:, t:t + 1],
                            scalar2=None, op0=ALU.mult)
                    else:
                        nc.vector.tensor_copy(
                            out=s["dst"][:, t, st * QT:(st + 1) * QT], in_=s["ps"])
                    strip_queue.pop(0)
                nmm -= 1

        def kq_strip(t, w_sb, br_sb, dst, st):
            push_strip(t, w_sb, br_sb, dst, st)
            strip_step(10 ** 6)

        def v_group(kt):
            # v strips kt and kt+1 in one PSUM tile, one strided copy out
            ps = pbqk.tile([128, QT], f32, tag="qk", name=f"vg{kt}")
            # j-outer: each accumulation group completes before the next
            # starts (start=True clears has_written for the WHOLE bank)
            for j in range(2):
                for c in range(NDC):
                    nc.tensor.matmul(ps[:, j * CW:(j + 1) * CW],
                                     xt_sb[:, c,
                                           (kt + j) * 128:(kt + j + 1) * 128],
                                     wv_sb[:, c, :],
                                     start=(c == 0), stop=(c == NDC - 1))
            nc.vector.tensor_copy(
                out=v2_sb[:, kt:kt + 2, :]
                    .rearrange("p k (h x) -> p k h x", x=VW)[:, :, :, 0:DH],
                in_=ps.rearrange("p (k h x) -> p k h x", k=2, x=DH))

        # minimal pair-0 prefix: k strip 0 + q strip 0, interleaved per
        # d-chunk so both finish right after the last xt chunk DMA lands
        psk = pbqk.tile([128, QT], f32, tag="qk", name="pfx_k")
        psq = pbqk.tile([128, QT], f32, tag="qk", name="pfx_q")
        for c in range(NDC):
            nc.tensor.matmul(psk, wk_sb[:, c, 0:128], xt_sb[:, c, 0:QT],
                             start=(c == 0), stop=False)
            nc.tensor.matmul(psq, wq_sb[:, c, 0:128], xt_sb[:, c, 0:QT],
                             start=(c == 0), stop=False)
        nc.tensor.matmul(psk, bkr_sb[:, 0:128], onesrow_sb, start=False, stop=True)
        nc.tensor.matmul(psq, bqr_sb[:, 0:128], onesrow_sb, start=False, stop=True)
        nc.vector.tensor_scalar(out=kT_sb[:, 0, 0:QT], in0=psk,
                                scalar1=lamv_sb[:, 0:1], scalar2=None,
                                op0=ALU.mult)
        nc.vector.tensor_copy(out=qT_sb[:, 0, 0:QT], in_=psq)

        # strips to hide under each (t, qt) attention tile
        plan = {
            (0, 1): [(1, wk_sb, bkr_sb, kT_sb, 0), (1, wq_sb, bqr_sb, qT_sb, 0),
                     (0, wq_sb, bqr_sb, qT_sb, 2)],
            (0, 2): [(1, wk_sb, bkr_sb, kT_sb, 1), (1, wk_sb, bkr_sb, kT_sb, 2),
                     (0, wq_sb, bqr_sb, qT_sb, 3)],
            (0, 3): [(1, wk_sb, bkr_sb, kT_sb, 3), (1, wq_sb, bqr_sb, qT_sb, 1)],
            (1, 0): [(1, wq_sb, bqr_sb, qT_sb, 2)],
            (1, 1): [(1, wq_sb, bqr_sb, qT_sb, 3)],
        }

        pending_post = [None]
        pending_nrmg = []
        step_ctr = [0]

        def flush_nrmg(min_age):
            for item in list(pending_nrmg):
                if step_ctr[0] - item[0] >= min_age:
                    _, tt, hf, hsl = item
                    for g in range(4):
                        nc.sync.dma_start(out=nrmg_sb[:, 2 * g + tt, hsl],
                                          in_=ago[tt, hf][g])
                    pending_nrmg.remove(item)

        def make_post(t, qt, h0, h1, zs0, zs1, rb0, rb1):
            qsl = slice(qt * QT, (qt + 1) * QT)

            def post():
                nc.vector.reciprocal_approx_fast(rb0, rb0)
                nc.vector.reciprocal_approx_fast(rb1, rb1)
                nc.vector.tensor_mul(zp_sb[t][0:64, qsl], zs0[0:DH, :], rb0)
                zst = pd.tile([64, QT], bf16, tag="zst", bufs=2,
                              name=f"zst{t}{qt}")
                nc.vector.tensor_mul(zst, zs1[0:DH, :], rb1)
                nc.gpsimd.dma_start(out=zp_sb[t][64:128, qsl], in_=zst)
                nc.vector.bn_stats(out=bnst[2 * (h0 % 2) + h0 // 2][:, qt, :],
                                   in_=zp_sb[t][0:64, qsl])
                nc.vector.bn_stats(out=bnst[2 * (h1 % 2) + h1 // 2][:, qt, :],
                                   in_=zst)
                # gather half-chunk once both its qts are posted
                if qt % 2 == 1 and not (t == 1 and qt == 3):
                    hf = qt // 2
                    hsl = slice(hf * 2 * QT, (hf + 1) * 2 * QT)
                    nc.sync.dma_start(out=agi[t, hf][:, :], in_=zp_sb[t][:, hsl])
                    if with_collective:
                        nc.gpsimd.collective_compute(
                            "AllGather", ALU.bypass, replica_groups=groups,
                            ins=[agi[t, hf][:].opt()], outs=[ago[t, hf][:].opt()])
                    else:
                        for g in range(4):
                            nc.sync.dma_start(out=ago[t, hf][g],
                                              in_=agi[t, hf][:, :])
                    pending_nrmg.append([step_ctr[0], t, hf, hsl])
                if qt == 3:
                    # pair-end stats aggregation (parity-major slots)
                    for h in (h0, h1):
                        j = 2 * (h % 2) + h // 2
                        mvh = pd.tile([64, 2], f32, tag="mv", bufs=2,
                                      name=f"mv{h}")
                        nc.vector.bn_aggr(out=mvh, in_=bnst[j])
                        stk = stk_all[j]
                        nc.vector.tensor_add(stk[:, 0:1], mvh[:, 0:1],
                                             bv_sb[:, h:h + 1])
                        nc.vector.tensor_copy(stk[:, 1:2], mvh[:, 1:2])
                        nc.vector.tensor_mul(stk[:, 2:3], stk[:, 0:1],
                                             stk[:, 0:1])

            return post

        for t in range(2):
            h0, h1 = 2 * t, 2 * t + 1
            for qt in range(NQT):
                step_ctr[0] += 1
                flush_nrmg(2)
                # safety: nothing stale may remain queued once this tile's
                # scores (which may read strip outputs) are emitted
                strip_step(10 ** 6)
                for sp in plan.get((t, qt), []):
                    push_strip(*sp)
                qsl = slice(qt * QT, (qt + 1) * QT)
                av0 = pav.tile([VW, QT], f32, tag="av", name=f"av{t}{qt}a")
                av1 = pav.tile([VW, QT], f32, tag="av", name=f"av{t}{qt}b")
                for kt in range(16):
                    sps = psc.tile([128, 2 * QT], f32, tag="s", name=f"s{t}{qt}{kt}")
                    for o in range(2):
                        nc.tensor.matmul(
                            sps[:, o * QT:(o + 1) * QT],
                            kT_sb[64 * o:64 * (o + 1), t, kt * 128:(kt + 1) * 128],
                            qT_sb[64 * o:64 * (o + 1), t, qt * QT:(qt + 1) * QT],
                            start=True, stop=True)
                    e_sb = pexp.tile([128, 2 * QT], bf16, tag="e", name=f"e{t}{qt}{kt}")
                    if kt % 3 == 1 and _os.environ.get("DVEEXP", "0") == "1":
                        # Schraudolph 2^x in bf16 via int16 bit trick (DVE)
                        nc.vector.tensor_scalar(
                            out=e_sb[:, :].bitcast(mybir.dt.int16), in0=sps,
                            scalar1=128.0, scalar2=16250.5,
                            op0=ALU.mult, op1=ALU.add)
                    else:
                        nc.scalar.activation(e_sb, sps, AF.Exp,
                                             scale=0.6931471805599453)
                    if t == 0 and qt == 0:
                        if kt % 2 == 0:
                            v_group(kt)
                        if kt in (2, 6, 10):
                            kq_strip(0, wk_sb, bkr_sb, kT_sb, kt // 4 + 1)
                        if kt == 8:
                            push_strip(0, wq_sb, bqr_sb, qT_sb, 1)
                    if kt == 6 and pending_post[0] is not None:
                        pending_post[0]()
                        pending_post[0] = None
                    nc.tensor.matmul(av0, v2_sb[:, kt, h0 * VW:(h0 + 1) * VW],
                                     e_sb[:, 0:QT],
                                     start=(kt == 0), stop=(kt == NKT - 1))
                    nc.tensor.matmul(av1, v2_sb[:, kt, h1 * VW:(h1 + 1) * VW],
                                     e_sb[:, QT:2 * QT],
                                     start=(kt == 0), stop=(kt == NKT - 1))
                    strip_step(2)

                if pending_post[0] is not None:
                    pending_post[0]()
                    pending_post[0] = None
                # immediate part of the qt post: drain the AV PSUM and kick
                # off the denominator-broadcast DMA roundtrip in background
                zs0 = pd.tile([VW, QT], f32, tag="zs", bufs=4, name=f"zs{t}{qt}a")
                zs1 = pd.tile([VW, QT], f32, tag="zs", bufs=4, name=f"zs{t}{qt}b")
                nc.vector.tensor_copy(zs0, av0)
                nc.vector.tensor_copy(zs1, av1)
                nc.sync.dma_start(out=rs_d[h0:h0 + 1, qsl], in_=zs0[DH:DH + 1, :])
                nc.sync.dma_start(out=rs_d[h1:h1 + 1, qsl], in_=zs1[DH:DH + 1, :])
                rb0 = pd.tile([64, QT], f32, tag="rb", bufs=4, name=f"rb{t}{qt}a")
                rb1 = pd.tile([64, QT], f32, tag="rb", bufs=4, name=f"rb{t}{qt}b")
                nc.sync.dma_start(
                    out=rb0, in_=rs_d[h0:h0 + 1, qsl].to_broadcast([64, QT]))
                nc.sync.dma_start(
                    out=rb1, in_=rs_d[h1:h1 + 1, qsl].to_broadcast([64, QT]))
                pending_post[0] = make_post(t, qt, h0, h1, zs0, zs1, rb0, rb1)

        # flush the last post (pair-1 qt3) immediately — it is the tail path
        pending_post[0]()
        pending_post[0] = None
        step_ctr[0] += 2
        flush_nrmg(0)

        warm0 = pbqk.tile([128, QT], f32, tag="qk", name="warm0")
        for i in range(35):
            nc.tensor.matmul(warm0, wo_sb[:, i % NDC, 0:128],
                             qT_sb[:, 0, 0:QT], start=True, stop=True)

        # ---- GN scalar tail (after last exp; one sqrt table switch) ----
        stp = pbqk.tile([128, QT], f32, tag="qk", name="stp")
        nc.tensor.matmul(stp[0:1, 0:12], ones_sb, stk_big[:, :, :],
                         start=True, stop=True)
        e3 = pd.tile([1, HPC, 3], f32, tag="e3", name="e3")
        nc.vector.tensor_scalar(
            out=e3, in0=stp[0:1, 0:12].rearrange("p (h x) -> p h x", x=3),
            scalar1=1.0 / 64.0, scalar2=None, op0=ALU.mult)
        m2 = pd.tile([1, HPC], f32, tag="m2", name="m2")
        nc.vector.tensor_mul(m2, e3[:, :, 0], e3[:, :, 0])
        nc.vector.tensor_add(vr_all, e3[:, :, 1], e3[:, :, 2])
        nc.vector.tensor_tensor(out=vr_all, in0=vr_all, in1=m2, op=ALU.subtract)
        eps_t = pd.tile([1, 1], f32, tag="eps", name="eps_t")
        nc.vector.memset(eps_t, EPS)
        sd_all = pd.tile([1, HPC], f32, tag="sd", name="sd_all")
        nc.scalar.activation(sd_all, vr_all, AF.Sqrt, bias=eps_t)
        rr = pd.tile([1, HPC], f32, tag="rr", name="rr")
        nc.vector.reciprocal(rr, sd_all)
        # stats already parity-major: payload = [M0,M2,M1,M3, r0,r2,r1,r3]
        nc.vector.tensor_copy(msc_all[:, 0:HPC], e3[:, :, 0])
        nc.vector.tensor_copy(msc_all[:, HPC:2 * HPC], rr)

        # scalars ride the last gather chunk as bitcast bf16 payload columns
        warm = pbqk.tile([128, QT], f32, tag="qk", name="warm")
        for i in range(100):
            nc.tensor.matmul(warm, wo_sb[:, i % NDC, 0:128],
                             qT_sb[:, 0, 0:QT], start=True, stop=True)

        nc.vector.tensor_copy(out=zp_sb[1][0:1, S:S + SCC],
                              in_=msc_all[0:1, :].bitcast(bf16))
        nc.sync.dma_start(out=agi[1, 1][:, :], in_=zp_sb[1][:, 2 * QT:S + SCC])
        if with_collective:
            nc.gpsimd.collective_compute(
                "AllGather", ALU.bypass, replica_groups=groups,
                ins=[agi[1, 1][:].opt()], outs=[ago[1, 1][:].opt()])
        else:
            for g in range(4):
                nc.sync.dma_start(out=ago[1, 1][g], in_=agi[1, 1][:, :])
        for g in range(4):
            nc.sync.dma_start(out=nrmg_sb[:, 2 * g + 1, 2 * QT:S],
                              in_=ago[1, 1][g, :, 0:2 * QT])

        cctx.close()

        # ---- fold GN affine into Wo; column-parallel out-projection ----
        with tc.tile_pool(name="pg", bufs=1) as pg, \
             tc.tile_pool(name="pf", bufs=4, space="PSUM") as pf, \
             tc.tile_pool(name="pystage", bufs=4) as pystage:
            # gathered scalars: [4 groups, 8 f32] as bitcast bf16 rows
            sc16 = pg.tile([1, 4, SCC], bf16, tag="sc16")
            nc.sync.dma_start(
                out=sc16,
                in_=ago[1, 1][:, 0:1, 2 * QT:2 * QT + SCC].rearrange("g p c -> p g c"))
            scf = sc16[:, :, :].bitcast(f32)   # [1, 4, 8]: (M0,M2,M1,M3, r0,r2,r1,r3)
            s2p = pf.tile([128, NDC], f32, tag="sp", bufs=1, name="s2p")
            mcp = pf.tile([128, NDC], f32, tag="mc", bufs=1, name="mcp")
            for o in range(2):
                nc.tensor.matmul(s2p[64 * o:64 * (o + 1), :], ones2_sb[0:1, :],
                                 scf[:, :, HPC + 2 * o:HPC + 2 * o + 2],
                                 start=True, stop=True)
                nc.tensor.matmul(mcp[64 * o:64 * (o + 1), :], ones2_sb[0:1, :],
                                 scf[:, :, 2 * o:2 * o + 2],
                                 start=True, stop=True)
            s2c = pg.tile([128, NDC], f32, tag="s2c")
            nc.vector.tensor_copy(s2c, s2p)
            bvg = pg.tile([128, NDC], f32, tag="bvg")
            nc.sync.dma_start(out=bvg, in_=bvf_d[:].rearrange("(c p) -> p c", p=128))
            mcs = pg.tile([128, NDC], f32, tag="mcs")
            nc.vector.tensor_tensor(out=mcs, in0=mcp, in1=bvg, op=ALU.subtract)
            mvec = pg.tile([128, NDC], bf16, tag="mvec")
            nc.vector.tensor_mul(mvec, mcs, s2c)

            # wo_scaled[p, (c,n)] = r_head(p,c) * wo ; cst[n] = sum_p M*r*wo
            wos = pg.tile([128, NDC, CW], bf16, tag="wos")
            for c in range(NDC):
                nc.vector.tensor_scalar(out=wos[:, c, :], in0=wo_sb[:, c, :],
                                        scalar1=s2c[:, c:c + 1], scalar2=None,
                                        op0=ALU.mult)
            cstp = pf.tile([128, QT], f32, tag="cs", bufs=1, name="cstp")
            for c in range(NDC):
                nc.tensor.matmul(cstp[0:1, 0:CW], mvec[:, c:c + 1], wo_sb[:, c, :],
                                 start=(c == 0), stop=(c == NDC - 1))
            brow = pg.tile([1, CW], bf16, tag="brow")
            nc.vector.tensor_tensor(out=brow, in0=bor_sb, in1=cstp[0:1, 0:CW],
                                    op=ALU.subtract)

            # per-st pipelined out-projection: 8 chunk matmuls + bias row
            for st in range(NQT):
                ssl = slice(st * QT, (st + 1) * QT)
                for nt in range(2):
                    yps = pf.tile([128, QT], f32, tag="yp", name=f"yp{nt}{st}")
                    for c in range(NDC):
                        nc.tensor.matmul(yps, wos[:, c, nt * 128:(nt + 1) * 128],
                                         nrmg_sb[:, c, ssl],
                                         start=(c == 0), stop=False)
                    nc.tensor.matmul(yps, brow[:, nt * 128:(nt + 1) * 128],
                                     onesrow_sb, start=False, stop=True)
                    ystage = pystage.tile([128, QT], f32, tag="ys",
                                          name=f"ys{nt}{st}")
                    nc.vector.tensor_copy(ystage, yps)
                    nc.sync.dma_start(out=y_d[nt, :, ssl], in_=ystage)

    nc.compile()
    return nc


def _get_nc():
    if "nc" not in _cache:
        _cache["nc"] = _build()
    return _cache["nc"]


def _host_prep(x, Wq, bq, Wk, bk, Wv, bv, Wo, bo, lq1, lk1, lq2, lk2, gn_w, gn_b):
    x = np.asarray(x, np.float32)
    lam = (np.exp((np.asarray(lq1) * np.asarray(lk1)).sum(-1))
           - np.exp((np.asarray(lq2) * np.asarray(lk2)).sum(-1)) + LAMBDA_INIT)
    qscale = (DH ** -0.5) * lam * np.log2(np.e)
    gw = np.asarray(gn_w).reshape(D)
    gb = np.asarray(gn_b).reshape(D)
    Wo_eff = np.asarray(Wo) * gw[:, None]
    bo_eff = np.asarray(bo) + gb @ np.asarray(Wo)

    # Gathered-row order (chunk (g t), partition (o,dh) -> head 4g+2t+o) is
    # exactly the original row-major head order, so Wo_eff rows need no
    # permutation.
    xT = np.ascontiguousarray(x.transpose(0, 2, 1))  # [B, D, S]
    bf = ml_dtypes.bfloat16

    in_maps = []
    for c in range(N_CORES):
        b, hg = c // 4, c % 4
        cs = slice(CW * hg, CW * (hg + 1))
        lamv = np.empty((128, 2), np.float32)
        for t in range(2):
            lamv[0:64, t] = qscale[4 * hg + 2 * t]
            lamv[64:128, t] = qscale[4 * hg + 2 * t + 1]
        in_maps.append({
            "xt": np.ascontiguousarray(xT[b]).astype(bf),
            "wq": _wlayout(np.asarray(Wq)[:, cs]).astype(bf),
            "wk": _wlayout(np.asarray(Wk)[:, cs]).astype(bf),
            "wv": _wlayout(np.asarray(Wv)[:, cs]).astype(bf),
            "lamv": lamv,
            "wo": _wlayout(Wo_eff[:, cs]).astype(bf),
            "bq": np.ascontiguousarray(np.asarray(bq)[cs]).astype(bf),
            "bk": np.ascontiguousarray(np.asarray(bk)[cs]).astype(bf),
            "bv": np.ascontiguousarray(np.asarray(bv)[cs]).astype(np.float32),
            "bvf": np.ascontiguousarray(np.asarray(bv)).astype(np.float32),
            "bo": np.ascontiguousarray(bo_eff[cs]).astype(bf),
        })
    return in_maps


def _wlayout(w):
    # [D, CW] -> [128, NDC, CW] so the on-device weight DMA is contiguous
    return np.ascontiguousarray(w.reshape(NDC, 128, CW).transpose(1, 0, 2))


def _host_gather(outs):
    # core c=4b+hg produced output columns [256*hg, 256*(hg+1)) as [2,128,S]
    yT = np.empty((B, D, S), np.float32)
    for b in range(B):
        for hg in range(4):
            q = np.asarray(outs[4 * b + hg]["y"]).reshape(CW, S)
            yT[b, CW * hg:CW * (hg + 1), :] = q
    return np.ascontiguousarray(yT.transpose(0, 2, 1))


def kernel(x, Wq, bq, Wk, bk, Wv, bv, Wo, bo, lq1, lk1, lq2, lk2, gn_w, gn_b):
    from concourse.bass_utils import run_bass_kernel_spmd

    in_maps = _host_prep(x, Wq, bq, Wk, bk, Wv, bv, Wo, bo,
                         lq1, lk1, lq2, lk2, gn_w, gn_b)
    nc = _get_nc()
    res = run_bass_kernel_spmd(nc, in_maps, core_ids=list(range(N_CORES)))
    return _host_gather(res.results)
